# revision 21
# baseline (speedup 1.0000x reference)
"""Distributed Trainium2 (8 NeuronCores) kernel for a 2-layer GCN actor-critic.

Math (PyG GCNConv with self-loops, norm by dst-concat degree):
    out = D (A + I) D X W + b,   D = diag(1/sqrt(1 + in_deg))
Reordered to aggregate FIRST over raw node features:
    M = (A + I) D X          (sparse aggregation, values dinv[src])
    out = relu(D (M W) + b)

Device mapping per core (dst-node partition, 6250 dst/core):
  - x_full (node-major bf16) host-staged in every core's DRAM; layer-1
    aggregation starts immediately (no transpose / all-gather first).
  - Aggregation: gpsimd dma_gather of source rows into SBUF edge-slot
    tiles (int16 indices; sources split into two streams by source core
    0-3 / 4-7 so indices fit int16; negative indices skip pad slots),
    then a one-hot "B-matmul" scatter on the TensorEngine: per 128-slot
    block, psum[f, dstcols] += G_block[slots, f]^T @ B_block[slots, cols],
    B carrying dinv[src] (bf16), zeros on pad slots.
  - Destination columns are degree-sorted per core; a global "staircase"
    slot map (max over cores -> identical SPMD program) gives column c
    exactly L[c] slots per stream.
  - Per 128-column window: accumulate agg in PSUM (feature-major, DVE
    pre-zeroed, start=False), copy to SBUF bf16, W-matmul, fused epilogue
    (x dinv, +b, relu) -> X^T bf16.
  - Between layers: PE-transpose windows to node-major, DMA to DRAM,
    AllGather -> layer-2 gather table (the stream split by source core is
    layer-invariant, so slot layout/B are shared between layers).
  - Heads fused per window: [logits; value] via M=2 matmul.
"""

import os
import sys

for _p in ("/opt/trn_rl_repo",):
    if _p not in sys.path:
        sys.path.insert(0, _p)

import numpy as np
import ml_dtypes

import concourse.bass as bass
import concourse.mybir as mybir
from concourse import tile
import concourse.bacc as bacc
from concourse.bass_utils import run_bass_kernel_spmd

BF16 = ml_dtypes.bfloat16
NC_CORES = 8
F = 256
P = 128
TCALL = 16  # gather blocks per dma_gather call
AG_CHUNKS = 8

LAST_EXEC_NS = None


def _plan(n_nodes, src, dst):
    """Static plan shared by all cores + per-core helpers."""
    npc = (n_nodes + NC_CORES - 1) // NC_CORES
    cols = ((npc + P - 1) // P) * P
    n_win = cols // P
    half_core = NC_CORES // 2

    deg_in = np.bincount(dst, minlength=n_nodes).astype(np.int64)
    half_nodes = half_core * npc
    # stream of a SOURCE node: fixed by original id (low half -> stream A).
    stream_of_node = (np.arange(n_nodes) >= half_nodes).astype(np.int64)

    # per-dst-node slot counts per stream (incl. self-loop), graph-fixed
    cntA = np.bincount(dst[stream_of_node[src] == 0], minlength=n_nodes)
    cntB = np.bincount(dst[stream_of_node[src] == 1], minlength=n_nodes)
    cntA = cntA + (stream_of_node == 0)
    cntB = cntB + (stream_of_node == 1)

    # deal low-half nodes to cores 0..half-1, high-half to cores half..,
    # sorted by (slotsA, slotsB) desc so column c holds near-equal counts
    core_of = np.zeros(n_nodes, dtype=np.int64)
    node_to_col = np.zeros(n_nodes, dtype=np.int64)
    col_to_node = [None] * NC_CORES
    for lo, hi, cbase in ((0, half_nodes, 0), (half_nodes, n_nodes, half_core)):
        ids = np.arange(lo, min(hi, n_nodes))
        order = np.lexsort((-cntB[ids], -cntA[ids]))
        ids = ids[order]
        ncg = half_core
        for k in range(ncg):
            sel = ids[k::ncg]
            core_of[sel] = cbase + k
            node_to_col[sel] = np.arange(len(sel))
            col_to_node[cbase + k] = sel

    # per-stream per-col counts, maxed over cores (incl. self-loop stream)
    Ls = np.zeros((2, cols), dtype=np.int64)
    src_stream = stream_of_node[src]
    for c in range(NC_CORES):
        mask = core_of[dst] == c
        ecol = node_to_col[dst[mask]]
        estr = src_stream[mask]
        ids = col_to_node[c]
        self_s = stream_of_node[ids]
        for s_ in (0, 1):
            cnt = np.bincount(ecol[estr == s_], minlength=cols)
            cnt[: len(ids)] += (self_s == s_)
            Ls[s_] = np.maximum(Ls[s_], cnt)

    # window-aligned staircase per stream; A-region and B-region slot spaces
    slot_start = np.zeros((2, cols), dtype=np.int64)  # region-local slot of col start
    col_of_slot = [[], []]
    win_blocks = []  # per window: list of (stream, region-local block id)
    reg_blocks = [0, 0]
    for w in range(n_win):
        wcols = np.arange(w * P, (w + 1) * P)
        entry = []
        for s_ in (0, 1):
            ls = Ls[s_][wcols]
            base = reg_blocks[s_] * P
            slot_start[s_][wcols] = base + np.concatenate([[0], np.cumsum(ls)[:-1]])
            used = int(ls.sum())
            k_w = max(1, (used + P - 1) // P)
            cos = np.full(k_w * P, -1, dtype=np.int64)
            reps = np.repeat(wcols, ls)
            cos[:used] = reps
            cos[used:] = reps[-1] if used > 0 else wcols[0]
            col_of_slot[s_].append(cos)
            entry.extend((s_, reg_blocks[s_] + b) for b in range(k_w))
            reg_blocks[s_] += k_w
        win_blocks.append(entry)
    col_of_slot = [np.concatenate(c_) for c_ in col_of_slot]
    t_reg = [reg_blocks[0], reg_blocks[1]]
    t_reg_pad = [((t + TCALL - 1) // TCALL) * TCALL for t in t_reg]

    # per-(stream, block) metadata + packed-B layout in processing order
    c0 = [np.zeros(t_reg[s_], dtype=np.int64) for s_ in (0, 1)]
    span = [np.zeros(t_reg[s_], dtype=np.int64) for s_ in (0, 1)]
    for s_ in (0, 1):
        for t in range(t_reg[s_]):
            cs = col_of_slot[s_][t * P:(t + 1) * P]
            c0[s_][t] = cs.min()
            span[s_][t] = cs.max() - c0[s_][t] + 1
    boff = [np.zeros(t_reg[s_], dtype=np.int64) for s_ in (0, 1)]
    off = 0
    for w in range(n_win):
        for (s_, t) in win_blocks[w]:
            boff[s_][t] = off
            off += span[s_][t]
    spans_total = int(off)

    return dict(
        npc=npc, cols=cols, n_win=n_win, half_core=half_core,
        t_reg=t_reg, t_reg_pad=t_reg_pad, Ls=Ls, slot_start=slot_start,
        win_blocks=win_blocks, col_of_slot=col_of_slot, c0=c0, span=span,
        boff=boff, spans_total=spans_total, col_to_node=col_to_node,
        node_to_col=node_to_col, core_of=core_of, deg_in=deg_in,
        stream_of_node=stream_of_node,
    )


def _wrap16(vals, t_pad):
    """Slot-sequential int16 values -> dma_gather wrapped layout [128, t_pad*8]."""
    n = t_pad * P
    a = np.full(n, -1, dtype=np.int16)
    a[: len(vals)] = vals
    a = a.reshape(n // 16, 16).T  # [16, n/16]
    return np.ascontiguousarray(np.tile(a, (8, 1)))


def _per_core_data(plan, n_nodes, src, dst, dinv):
    cols = plan["cols"]
    col_of_slot, c0, boff = plan["col_of_slot"], plan["c0"], plan["boff"]
    slot_start, node_to_col, core_of = plan["slot_start"], plan["node_to_col"], plan["core_of"]
    stream_of_node = plan["stream_of_node"]
    t_reg, t_reg_pad = plan["t_reg"], plan["t_reg_pad"]
    spans_total = plan["spans_total"]
    half1 = plan["half_core"] * plan["npc"]   # layer-1 table split (node id half)
    half2 = plan["half_core"] * cols          # layer-2 table split (gathered row)

    dinv_bf = dinv.astype(BF16)
    out = []
    for c in range(NC_CORES):
        mask = core_of[dst] == c
        s_c, d_c = src[mask], dst[mask]
        ecol = node_to_col[d_c]
        estr = stream_of_node[s_c]
        ids = plan["col_to_node"][c]

        bvals = np.zeros((P, spans_total), dtype=BF16)
        idx1 = []
        idx2 = []
        for s_ in (0, 1):
            nslot = t_reg[s_] * P
            m2 = estr == s_
            sc2, col2 = s_c[m2], ecol[m2]
            order = np.argsort(col2, kind="stable")
            sc2, col2 = sc2[order], col2[order]
            counts = np.bincount(col2, minlength=cols)
            starts = np.concatenate([[0], np.cumsum(counts)[:-1]])
            rank = np.arange(len(col2)) - starts[col2]
            eslot = slot_start[s_][col2] + rank

            src_flat = np.full(nslot, -1, dtype=np.int64)
            src_flat[eslot] = sc2
            sids = ids[stream_of_node[ids] == s_]
            scol = node_to_col[sids]
            self_slot = slot_start[s_][scol] + counts[scol]
            src_flat[self_slot] = sids

            valid = src_flat >= 0
            # forward/backward-fill pad slots with a neighbouring real source
            # (dma_gather forbids mid-stream negatives; B weight is 0 there)
            ffill = np.arange(nslot)
            ffill[~valid] = 0
            ffill = np.maximum.accumulate(ffill)
            fsrc = src_flat[ffill]
            if fsrc[0] < 0:  # leading pads: backward-fill
        # NOTE: backward fill via reversed accumulate
                rix = np.arange(nslot)
                rvalid = fsrc >= 0
                rr = np.where(rvalid, np.arange(nslot), nslot - 1)
                rr = np.minimum.accumulate(rr[::-1])[::-1]
                fsrc = src_flat[ffill]
                fsrc = np.where(rvalid, fsrc, src_flat[rr])
            assert (fsrc >= 0).all()
            vsrc = fsrc
            i1 = (vsrc - s_ * half1).astype(np.int16)
            g2 = core_of[vsrc] * cols + node_to_col[vsrc]
            i2 = (g2 - s_ * half2).astype(np.int16)
            idx1.append(_wrap16(i1, t_reg_pad[s_]))
            idx2.append(_wrap16(i2, t_reg_pad[s_]))

            s_all = np.arange(nslot)[valid]
            t_of = s_all // P
            bcol = boff[s_][t_of] + (col_of_slot[s_][s_all] - c0[s_][t_of])
            bvals[s_all % P, bcol] = dinv_bf[src_flat[valid]]

        dinvb = np.zeros((P, cols), dtype=np.float32)
        dinvb[:, : len(ids)] = dinv[ids][None, :]
        out.append(dict(iA1=idx1[0], iB1=idx1[1], iA2=idx2[0], iB2=idx2[1],
                        bvals=bvals, dinvb=dinvb))
    return out


def _build(plan, n_nodes):
    cols, n_win = plan["cols"], plan["n_win"]
    t_reg, t_reg_pad = plan["t_reg"], plan["t_reg_pad"]
    spans_total = plan["spans_total"]
    win_blocks, c0, span, boff = plan["win_blocks"], plan["c0"], plan["span"], plan["boff"]
    half1 = plan["half_core"] * plan["npc"]
    half2 = plan["half_core"] * cols
    n_calls = [t // TCALL for t in t_reg_pad]

    f32, bf16, i16 = mybir.dt.float32, mybir.dt.bfloat16, mybir.dt.int16
    nc = bacc.Bacc(None, target_bir_lowering=False, debug=False,
                   num_swdge_queues=4)

    xfull = nc.declare_dram_parameter("xfull", [n_nodes, F], bf16, isOutput=False)
    w1p = nc.declare_dram_parameter("w1p", [P, 2 * F], bf16, isOutput=False)
    w2p = nc.declare_dram_parameter("w2p", [P, 2 * F], bf16, isOutput=False)
    wacp = nc.declare_dram_parameter("wacp", [P, 4], bf16, isOutput=False)
    b1p = nc.declare_dram_parameter("b1p", [P, 2], f32, isOutput=False)
    b2p = nc.declare_dram_parameter("b2p", [P, 2], f32, isOutput=False)
    bacp = nc.declare_dram_parameter("bacp", [2, 1], f32, isOutput=False)
    dinvp = nc.declare_dram_parameter("dinvp", [P, cols], f32, isOutput=False)
    bvp = nc.declare_dram_parameter("bvp", [P, spans_total], bf16, isOutput=False)
    ixp = {}
    for lay in (1, 2):
        for s_ in (0, 1):
            nm = f"i{'AB'[s_]}{lay}"
            ixp[(lay, s_)] = nc.declare_dram_parameter(
                nm, [P, t_reg_pad[s_] * 8], i16, isOutput=False)
    outp = nc.declare_dram_parameter("out", [2, cols], f32, isOutput=True)

    with tile.TileContext(nc) as tc:
        with tc.tile_pool(name="dram", bufs=1, space="DRAM") as dram, \
             tc.tile_pool(name="const", bufs=1) as cst, \
             tc.tile_pool(name="gp", bufs=6) as gp, \
             tc.tile_pool(name="mp", bufs=2) as mp, \
             tc.tile_pool(name="xp", bufs=2) as xp, \
             tc.tile_pool(name="psA", bufs=2, space="PSUM") as psA, \
             tc.tile_pool(name="psW", bufs=1, space="PSUM") as psW, \
             tc.tile_pool(name="psM", bufs=2, space="PSUM") as psM:

            ag_in = dram.tile([cols, F], bf16, name="ag_in")
            ag_out = dram.tile([NC_CORES * cols, F], bf16, name="ag_out")
            wb = [0] + [((n_win * (ci + 1)) // AG_CHUNKS) for ci in range(AG_CHUNKS)]
            agc = [dram.tile([NC_CORES * (wb[ci + 1] - wb[ci]) * P, F], bf16,
                             name=f"agc_{ci}") for ci in range(AG_CHUNKS)]

            idx = {}
            for s_ in (0, 1):
                idx[s_] = cst.tile([P, t_reg_pad[s_] * 8], i16, name=f"idx{s_}")
            bv = cst.tile([P, spans_total], bf16, name="bv")
            w1t = cst.tile([P, 2, F], bf16, name="w1t")
            w2t = cst.tile([P, 2, F], bf16, name="w2t")
            wact = cst.tile([P, 2, 2], bf16, name="wact")
            b1t = cst.tile([P, 2], f32, name="b1t")
            b2t = cst.tile([P, 2], f32, name="b2t")
            bact = cst.tile([2, 1], f32, name="bact")
            dinvt = cst.tile([P, cols], f32, name="dinvt")
            identf = cst.tile([P, P], bf16, name="identf")
            outt = cst.tile([2, cols], f32, name="outt")

            for s_ in (0, 1):
                nc.sync.dma_start(idx[s_][:], ixp[(1, s_)][:])
            nc.sync.dma_start(bv[:], bvp[:])
            nc.sync.dma_start(w1t[:].rearrange("p a f -> p (a f)"), w1p[:])
            nc.sync.dma_start(w2t[:].rearrange("p a f -> p (a f)"), w2p[:])
            nc.sync.dma_start(wact[:].rearrange("p a f -> p (a f)"), wacp[:])
            nc.sync.dma_start(b1t[:], b1p[:])
            nc.sync.dma_start(b2t[:], b2p[:])
            nc.sync.dma_start(bact[:], bacp[:])
            nc.sync.dma_start(dinvt[:], dinvp[:])
            # identity for PE transpose
            nc.gpsimd.memset(identf[:], 0.0)
            nc.gpsimd.affine_select(
                out=identf[:], in_=identf[:],
                compare_op=mybir.AluOpType.not_equal, fill=1.0,
                base=0, pattern=[[-1, P]], channel_multiplier=1)

            for layer in (1, 2):
                if layer == 2:
                    for s_ in (0, 1):
                        nc.sync.dma_start(idx[s_][:], ixp[(2, s_)][:])
                gtiles = {0: [None] * n_calls[0], 1: [None] * n_calls[1]}
                issued = {0: -1, 1: -1}

                def issue_call(s_, k, layer=layer, gtiles=gtiles, issued=issued,
                               t_reg=t_reg):
                    while issued[s_] < k:
                        kk = issued[s_] + 1
                        g = gp.tile([P, TCALL, F], bf16, tag=f"g{s_}",
                                    name=f"g_{layer}_{s_}_{kk}")
                        if layer == 1 and kk < 2:
                            nc.vector.memset(g[:], 0.0)
                        if layer == 1:
                            tab = xfull[0:half1, :] if s_ == 0 else \
                                  xfull[half1:n_nodes, :]
                        else:
                            tab = ag_out[0:half2, :] if s_ == 0 else \
                                  ag_out[half2:NC_CORES * cols, :]
                        nvalid = min(t_reg[s_] * P - kk * TCALL * P, TCALL * P)
                        nc.gpsimd.dma_gather(
                            g[:], tab,
                            idx[s_][:, kk * TCALL * 8:(kk + 1) * TCALL * 8],
                            num_idxs=TCALL * P, num_idxs_reg=nvalid,
                            elem_size=F, single_packet=False,
                            queue_num=2 * s_ + (kk % 2),
                        )
                        gtiles[s_][kk] = g
                        issued[s_] = kk

                wt = w1t if layer == 1 else w2t
                bt = b1t if layer == 1 else b2t
                for w in range(n_win):
                    pa = [psA.tile([P, P], f32, tag=f"agg{h}",
                                   name=f"agg_{layer}_{w}_{h}") for h in (0, 1)]
                    for h in (0, 1):
                        nc.vector.memset(pa[h][:], 0.0)
                    blocks = win_blocks[w]
                    for j, (s_, t) in enumerate(blocks):
                        issue_call(s_, t // TCALL)
                        g = gtiles[s_][t // TCALL]
                        tl = t % TCALL
                        a, b_ = int(c0[s_][t] - w * P), int(span[s_][t])
                        for h in (0, 1):
                            nc.tensor.matmul(
                                pa[h][:, a:a + b_],
                                lhsT=g[:, tl, h * P:(h + 1) * P],
                                rhs=bv[:, int(boff[s_][t]):int(boff[s_][t] + b_)],
                                start=False,
                                stop=(j == len(blocks) - 1),
                                skip_group_check=True,
                            )
                    m_sb = [mp.tile([P, P], bf16, tag=f"m{h}",
                                    name=f"m_{layer}_{w}_{h}") for h in (0, 1)]
                    for h in (0, 1):
                        nc.vector.tensor_tensor(
                            m_sb[h][:], pa[h][:], dinvt[:, w * P:(w + 1) * P],
                            mybir.AluOpType.mult)
                    pw = [psW.tile([P, P], f32, tag=f"w{h}",
                                   name=f"pw_{layer}_{w}_{h}") for h in (0, 1)]
                    for oh in (0, 1):
                        for kh in (0, 1):
                            nc.tensor.matmul(
                                pw[oh][:],
                                lhsT=wt[:, kh, oh * P:(oh + 1) * P],
                                rhs=m_sb[kh][:],
                                start=(kh == 0), stop=(kh == 1),
                            )
                    xt = xp.tile([P, 2, P], bf16, tag="xt", name=f"xt_{layer}_{w}")
                    for oh in (0, 1):
                        nc.vector.tensor_scalar(
                            xt[:, oh, :], pw[oh][:], bt[:, oh:oh + 1], 0.0,
                            mybir.AluOpType.add, mybir.AluOpType.max)
                    if layer == 1:
                        xn = xp.tile([P, F], bf16, tag="xn", name=f"xn_{w}")
                        for oh in (0, 1):
                            pt = psM.tile([P, P], bf16, tag="misc",
                                          name=f"pt_{w}_{oh}")
                            nc.tensor.transpose(pt[:], xt[:, oh, :], identf[:])
                            nc.vector.tensor_copy(xn[:, oh * P:(oh + 1) * P], pt[:])
                        nc.sync.dma_start(ag_in[w * P:(w + 1) * P, :], xn[:])
                        for ci in range(AG_CHUNKS):
                            if w + 1 == wb[ci + 1] and wb[ci] < wb[ci + 1]:
                                lo, hi = wb[ci] * P, wb[ci + 1] * P
                                nc.gpsimd.collective_compute(
                                    "AllGather", mybir.AluOpType.bypass,
                                    replica_groups=[list(range(NC_CORES))],
                                    ins=[ag_in[lo:hi, :].opt()],
                                    outs=[agc[ci][:].opt()],
                                )
                                nr = hi - lo
                                for r in range(NC_CORES):
                                    nc.scalar.dma_start(
                                        ag_out[r * cols + lo:r * cols + hi, :],
                                        agc[ci][r * nr:(r + 1) * nr, :])
                    else:
                        ph = psM.tile([2, P], f32, tag="misc", name=f"ph_{w}")
                        for kh in (0, 1):
                            nc.tensor.matmul(
                                ph[:], lhsT=wact[:, kh, :], rhs=xt[:, kh, :],
                                start=(kh == 0), stop=(kh == 1))
                        nc.vector.tensor_scalar(
                            outt[:, w * P:(w + 1) * P], ph[:], bact[:, 0:1], None,
                            mybir.AluOpType.add)

            nc.sync.dma_start(outp[:], outt[:])
    nc.compile()
    return nc


def kernel(x, edge_index, W1, b1, W2, b2, Wa, ba, Wc, bc):
    global LAST_EXEC_NS
    x = np.asarray(x)
    edge_index = np.asarray(edge_index)
    n_nodes = x.shape[0]
    src = edge_index[0].astype(np.int64)
    dst = edge_index[1].astype(np.int64)

    plan = _plan(n_nodes, src, dst)
    dinv = (1.0 / np.sqrt(1.0 + plan["deg_in"].astype(np.float64))).astype(np.float32)
    cores = _per_core_data(plan, n_nodes, src, dst, dinv)
    nc = _build(plan, n_nodes)

    x_bf = np.ascontiguousarray(x.astype(BF16))
    W1 = np.asarray(W1, dtype=np.float32); W2 = np.asarray(W2, dtype=np.float32)
    Wa = np.asarray(Wa, dtype=np.float32).reshape(F, 1)
    Wc = np.asarray(Wc, dtype=np.float32).reshape(F, 1)

    def pack_w(w):
        m = w.shape[1]
        return np.ascontiguousarray(
            w.reshape(2, P, m).transpose(1, 0, 2).reshape(P, 2 * m).astype(BF16))

    w1p, w2p = pack_w(W1), pack_w(W2)
    wacp = pack_w(np.concatenate([Wa, Wc], axis=1))

    def pack_b(b):
        return np.ascontiguousarray(np.asarray(b, np.float32).reshape(2, P).T)

    b1p, b2p = pack_b(b1), pack_b(b2)
    bacp = np.array([[float(np.asarray(ba).reshape(-1)[0])],
                     [float(np.asarray(bc).reshape(-1)[0])]], dtype=np.float32)

    in_maps = []
    for c in range(NC_CORES):
        d = cores[c]
        in_maps.append({
            "xfull": x_bf, "w1p": w1p, "w2p": w2p, "wacp": wacp,
            "b1p": b1p, "b2p": b2p, "bacp": bacp,
            "dinvp": d["dinvb"], "bvp": d["bvals"],
            "iA1": d["iA1"], "iB1": d["iB1"], "iA2": d["iA2"], "iB2": d["iB2"],
        })

    trace = bool(int(os.environ.get("KERNEL_TRACE", "0")))
    res = run_bass_kernel_spmd(nc, in_maps, core_ids=list(range(NC_CORES)),
                               trace=trace)
    LAST_EXEC_NS = res.exec_time_ns

    logits = np.zeros(n_nodes, dtype=np.float32)
    value = np.zeros((n_nodes, 1), dtype=np.float32)
    for c in range(NC_CORES):
        o = res.results[c]["out"]
        ids = plan["col_to_node"][c]
        logits[ids] = o[0, : len(ids)]
        value[ids, 0] = o[1, : len(ids)]
    return logits, value


# revision 22
# speedup vs baseline: 1.0098x; 1.0098x over previous
"""Distributed Trainium2 (8 NeuronCores) kernel for a 2-layer GCN actor-critic.

Math (PyG GCNConv with self-loops, norm by dst-concat degree):
    out = D (A + I) D X W + b,   D = diag(1/sqrt(1 + in_deg))
Reordered to aggregate FIRST over raw node features:
    M = (A + I) D X          (sparse aggregation, values dinv[src])
    out = relu(D (M W) + b)

Device mapping per core (dst-node partition, 6250 dst/core):
  - x_full (node-major bf16) host-staged in every core's DRAM; layer-1
    aggregation starts immediately (no transpose / all-gather first).
  - Aggregation: gpsimd dma_gather of source rows into SBUF edge-slot
    tiles (int16 indices; sources split into two streams by source core
    0-3 / 4-7 so indices fit int16; negative indices skip pad slots),
    then a one-hot "B-matmul" scatter on the TensorEngine: per 128-slot
    block, psum[f, dstcols] += G_block[slots, f]^T @ B_block[slots, cols],
    B carrying dinv[src] (bf16), zeros on pad slots.
  - Destination columns are degree-sorted per core; a global "staircase"
    slot map (max over cores -> identical SPMD program) gives column c
    exactly L[c] slots per stream.
  - Per 128-column window: accumulate agg in PSUM (feature-major, DVE
    pre-zeroed, start=False), copy to SBUF bf16, W-matmul, fused epilogue
    (x dinv, +b, relu) -> X^T bf16.
  - Between layers: PE-transpose windows to node-major, DMA to DRAM,
    AllGather -> layer-2 gather table (the stream split by source core is
    layer-invariant, so slot layout/B are shared between layers).
  - Heads fused per window: [logits; value] via M=2 matmul.
"""

import os
import sys

for _p in ("/opt/trn_rl_repo",):
    if _p not in sys.path:
        sys.path.insert(0, _p)

import numpy as np
import ml_dtypes

import concourse.bass as bass
import concourse.mybir as mybir
from concourse import tile
import concourse.bacc as bacc
from concourse.bass_utils import run_bass_kernel_spmd

BF16 = ml_dtypes.bfloat16
NC_CORES = 8
F = 256
P = 128
TCALL = 24  # gather blocks per dma_gather call
AG_CHUNKS = 8

LAST_EXEC_NS = None


def _plan(n_nodes, src, dst):
    """Static plan shared by all cores + per-core helpers."""
    npc = (n_nodes + NC_CORES - 1) // NC_CORES
    cols = ((npc + P - 1) // P) * P
    n_win = cols // P
    half_core = NC_CORES // 2

    deg_in = np.bincount(dst, minlength=n_nodes).astype(np.int64)
    half_nodes = half_core * npc
    # stream of a SOURCE node: fixed by original id (low half -> stream A).
    stream_of_node = (np.arange(n_nodes) >= half_nodes).astype(np.int64)

    # per-dst-node slot counts per stream (incl. self-loop), graph-fixed
    cntA = np.bincount(dst[stream_of_node[src] == 0], minlength=n_nodes)
    cntB = np.bincount(dst[stream_of_node[src] == 1], minlength=n_nodes)
    cntA = cntA + (stream_of_node == 0)
    cntB = cntB + (stream_of_node == 1)

    # deal low-half nodes to cores 0..half-1, high-half to cores half..,
    # sorted by (slotsA, slotsB) desc so column c holds near-equal counts
    core_of = np.zeros(n_nodes, dtype=np.int64)
    node_to_col = np.zeros(n_nodes, dtype=np.int64)
    col_to_node = [None] * NC_CORES
    for lo, hi, cbase in ((0, half_nodes, 0), (half_nodes, n_nodes, half_core)):
        ids = np.arange(lo, min(hi, n_nodes))
        order = np.lexsort((-cntB[ids], -cntA[ids]))
        ids = ids[order]
        ncg = half_core
        for k in range(ncg):
            sel = ids[k::ncg]
            core_of[sel] = cbase + k
            node_to_col[sel] = np.arange(len(sel))
            col_to_node[cbase + k] = sel

    # per-stream per-col counts, maxed over cores (incl. self-loop stream)
    Ls = np.zeros((2, cols), dtype=np.int64)
    src_stream = stream_of_node[src]
    for c in range(NC_CORES):
        mask = core_of[dst] == c
        ecol = node_to_col[dst[mask]]
        estr = src_stream[mask]
        ids = col_to_node[c]
        self_s = stream_of_node[ids]
        for s_ in (0, 1):
            cnt = np.bincount(ecol[estr == s_], minlength=cols)
            cnt[: len(ids)] += (self_s == s_)
            Ls[s_] = np.maximum(Ls[s_], cnt)

    # window-aligned staircase per stream; A-region and B-region slot spaces
    slot_start = np.zeros((2, cols), dtype=np.int64)  # region-local slot of col start
    col_of_slot = [[], []]
    win_blocks = []  # per window: list of (stream, region-local block id)
    reg_blocks = [0, 0]
    for w in range(n_win):
        wcols = np.arange(w * P, (w + 1) * P)
        entry = []
        for s_ in (0, 1):
            ls = Ls[s_][wcols]
            base = reg_blocks[s_] * P
            slot_start[s_][wcols] = base + np.concatenate([[0], np.cumsum(ls)[:-1]])
            used = int(ls.sum())
            k_w = max(1, (used + P - 1) // P)
            cos = np.full(k_w * P, -1, dtype=np.int64)
            reps = np.repeat(wcols, ls)
            cos[:used] = reps
            cos[used:] = reps[-1] if used > 0 else wcols[0]
            col_of_slot[s_].append(cos)
            entry.extend((s_, reg_blocks[s_] + b) for b in range(k_w))
            reg_blocks[s_] += k_w
        win_blocks.append(entry)
    col_of_slot = [np.concatenate(c_) for c_ in col_of_slot]
    t_reg = [reg_blocks[0], reg_blocks[1]]
    t_reg_pad = [((t + TCALL - 1) // TCALL) * TCALL for t in t_reg]

    # per-(stream, block) metadata + packed-B layout in processing order
    c0 = [np.zeros(t_reg[s_], dtype=np.int64) for s_ in (0, 1)]
    span = [np.zeros(t_reg[s_], dtype=np.int64) for s_ in (0, 1)]
    for s_ in (0, 1):
        for t in range(t_reg[s_]):
            cs = col_of_slot[s_][t * P:(t + 1) * P]
            c0[s_][t] = cs.min()
            span[s_][t] = cs.max() - c0[s_][t] + 1
    boff = [np.zeros(t_reg[s_], dtype=np.int64) for s_ in (0, 1)]
    off = 0
    for w in range(n_win):
        for (s_, t) in win_blocks[w]:
            boff[s_][t] = off
            off += span[s_][t]
    spans_total = int(off)

    return dict(
        npc=npc, cols=cols, n_win=n_win, half_core=half_core,
        t_reg=t_reg, t_reg_pad=t_reg_pad, Ls=Ls, slot_start=slot_start,
        win_blocks=win_blocks, col_of_slot=col_of_slot, c0=c0, span=span,
        boff=boff, spans_total=spans_total, col_to_node=col_to_node,
        node_to_col=node_to_col, core_of=core_of, deg_in=deg_in,
        stream_of_node=stream_of_node,
    )


def _wrap16(vals, t_pad):
    """Slot-sequential int16 values -> dma_gather wrapped layout [128, t_pad*8]."""
    n = t_pad * P
    a = np.full(n, -1, dtype=np.int16)
    a[: len(vals)] = vals
    a = a.reshape(n // 16, 16).T  # [16, n/16]
    return np.ascontiguousarray(np.tile(a, (8, 1)))


def _per_core_data(plan, n_nodes, src, dst, dinv):
    cols = plan["cols"]
    col_of_slot, c0, boff = plan["col_of_slot"], plan["c0"], plan["boff"]
    slot_start, node_to_col, core_of = plan["slot_start"], plan["node_to_col"], plan["core_of"]
    stream_of_node = plan["stream_of_node"]
    t_reg, t_reg_pad = plan["t_reg"], plan["t_reg_pad"]
    spans_total = plan["spans_total"]
    half1 = plan["half_core"] * plan["npc"]   # layer-1 table split (node id half)
    half2 = plan["half_core"] * cols          # layer-2 table split (gathered row)

    dinv_bf = dinv.astype(BF16)
    out = []
    for c in range(NC_CORES):
        mask = core_of[dst] == c
        s_c, d_c = src[mask], dst[mask]
        ecol = node_to_col[d_c]
        estr = stream_of_node[s_c]
        ids = plan["col_to_node"][c]

        bvals = np.zeros((P, spans_total), dtype=BF16)
        idx1 = []
        idx2 = []
        for s_ in (0, 1):
            nslot = t_reg[s_] * P
            m2 = estr == s_
            sc2, col2 = s_c[m2], ecol[m2]
            order = np.argsort(col2, kind="stable")
            sc2, col2 = sc2[order], col2[order]
            counts = np.bincount(col2, minlength=cols)
            starts = np.concatenate([[0], np.cumsum(counts)[:-1]])
            rank = np.arange(len(col2)) - starts[col2]
            eslot = slot_start[s_][col2] + rank

            src_flat = np.full(nslot, -1, dtype=np.int64)
            src_flat[eslot] = sc2
            sids = ids[stream_of_node[ids] == s_]
            scol = node_to_col[sids]
            self_slot = slot_start[s_][scol] + counts[scol]
            src_flat[self_slot] = sids

            valid = src_flat >= 0
            # forward/backward-fill pad slots with a neighbouring real source
            # (dma_gather forbids mid-stream negatives; B weight is 0 there)
            ffill = np.arange(nslot)
            ffill[~valid] = 0
            ffill = np.maximum.accumulate(ffill)
            fsrc = src_flat[ffill]
            if fsrc[0] < 0:  # leading pads: backward-fill
        # NOTE: backward fill via reversed accumulate
                rix = np.arange(nslot)
                rvalid = fsrc >= 0
                rr = np.where(rvalid, np.arange(nslot), nslot - 1)
                rr = np.minimum.accumulate(rr[::-1])[::-1]
                fsrc = src_flat[ffill]
                fsrc = np.where(rvalid, fsrc, src_flat[rr])
            assert (fsrc >= 0).all()
            vsrc = fsrc
            i1 = (vsrc - s_ * half1).astype(np.int16)
            g2 = core_of[vsrc] * cols + node_to_col[vsrc]
            i2 = (g2 - s_ * half2).astype(np.int16)
            idx1.append(_wrap16(i1, t_reg_pad[s_]))
            idx2.append(_wrap16(i2, t_reg_pad[s_]))

            s_all = np.arange(nslot)[valid]
            t_of = s_all // P
            bcol = boff[s_][t_of] + (col_of_slot[s_][s_all] - c0[s_][t_of])
            bvals[s_all % P, bcol] = dinv_bf[src_flat[valid]]

        dinvb = np.zeros((P, cols), dtype=np.float32)
        dinvb[:, : len(ids)] = dinv[ids][None, :]
        out.append(dict(iA1=idx1[0], iB1=idx1[1], iA2=idx2[0], iB2=idx2[1],
                        bvals=bvals, dinvb=dinvb))
    return out


def _build(plan, n_nodes):
    cols, n_win = plan["cols"], plan["n_win"]
    t_reg, t_reg_pad = plan["t_reg"], plan["t_reg_pad"]
    spans_total = plan["spans_total"]
    win_blocks, c0, span, boff = plan["win_blocks"], plan["c0"], plan["span"], plan["boff"]
    half1 = plan["half_core"] * plan["npc"]
    half2 = plan["half_core"] * cols
    n_calls = [t // TCALL for t in t_reg_pad]

    f32, bf16, i16 = mybir.dt.float32, mybir.dt.bfloat16, mybir.dt.int16
    nc = bacc.Bacc(None, target_bir_lowering=False, debug=False,
                   num_swdge_queues=4)

    xfull = nc.declare_dram_parameter("xfull", [n_nodes, F], bf16, isOutput=False)
    w1p = nc.declare_dram_parameter("w1p", [P, 2 * F], bf16, isOutput=False)
    w2p = nc.declare_dram_parameter("w2p", [P, 2 * F], bf16, isOutput=False)
    wacp = nc.declare_dram_parameter("wacp", [P, 4], bf16, isOutput=False)
    b1p = nc.declare_dram_parameter("b1p", [P, 2], f32, isOutput=False)
    b2p = nc.declare_dram_parameter("b2p", [P, 2], f32, isOutput=False)
    bacp = nc.declare_dram_parameter("bacp", [2, 1], f32, isOutput=False)
    dinvp = nc.declare_dram_parameter("dinvp", [P, cols], f32, isOutput=False)
    bvp = nc.declare_dram_parameter("bvp", [P, spans_total], bf16, isOutput=False)
    ixp = {}
    for lay in (1, 2):
        for s_ in (0, 1):
            nm = f"i{'AB'[s_]}{lay}"
            ixp[(lay, s_)] = nc.declare_dram_parameter(
                nm, [P, t_reg_pad[s_] * 8], i16, isOutput=False)
    outp = nc.declare_dram_parameter("out", [2, cols], f32, isOutput=True)

    with tile.TileContext(nc) as tc:
        with tc.tile_pool(name="dram", bufs=1, space="DRAM") as dram, \
             tc.tile_pool(name="const", bufs=1) as cst, \
             tc.tile_pool(name="gp", bufs=4) as gp, \
             tc.tile_pool(name="mp", bufs=2) as mp, \
             tc.tile_pool(name="xp", bufs=2) as xp, \
             tc.tile_pool(name="psA", bufs=2, space="PSUM") as psA, \
             tc.tile_pool(name="psW", bufs=1, space="PSUM") as psW, \
             tc.tile_pool(name="psM", bufs=2, space="PSUM") as psM:

            ag_in = dram.tile([cols, F], bf16, name="ag_in")
            ag_out = dram.tile([NC_CORES * cols, F], bf16, name="ag_out")
            wb = [0] + [((n_win * (ci + 1)) // AG_CHUNKS) for ci in range(AG_CHUNKS)]
            agc = [dram.tile([NC_CORES * (wb[ci + 1] - wb[ci]) * P, F], bf16,
                             name=f"agc_{ci}") for ci in range(AG_CHUNKS)]

            idx = {}
            for s_ in (0, 1):
                idx[s_] = cst.tile([P, t_reg_pad[s_] * 8], i16, name=f"idx{s_}")
            bv = cst.tile([P, spans_total], bf16, name="bv")
            w1t = cst.tile([P, 2, F], bf16, name="w1t")
            w2t = cst.tile([P, 2, F], bf16, name="w2t")
            wact = cst.tile([P, 2, 2], bf16, name="wact")
            b1t = cst.tile([P, 2], f32, name="b1t")
            b2t = cst.tile([P, 2], f32, name="b2t")
            bact = cst.tile([2, 1], f32, name="bact")
            dinvt = cst.tile([P, cols], f32, name="dinvt")
            identf = cst.tile([P, P], bf16, name="identf")
            outt = cst.tile([2, cols], f32, name="outt")

            for s_ in (0, 1):
                nc.sync.dma_start(idx[s_][:], ixp[(1, s_)][:])
            nc.sync.dma_start(bv[:], bvp[:])
            nc.sync.dma_start(w1t[:].rearrange("p a f -> p (a f)"), w1p[:])
            nc.sync.dma_start(w2t[:].rearrange("p a f -> p (a f)"), w2p[:])
            nc.sync.dma_start(wact[:].rearrange("p a f -> p (a f)"), wacp[:])
            nc.sync.dma_start(b1t[:], b1p[:])
            nc.sync.dma_start(b2t[:], b2p[:])
            nc.sync.dma_start(bact[:], bacp[:])
            nc.sync.dma_start(dinvt[:], dinvp[:])
            # identity for PE transpose
            nc.gpsimd.memset(identf[:], 0.0)
            nc.gpsimd.affine_select(
                out=identf[:], in_=identf[:],
                compare_op=mybir.AluOpType.not_equal, fill=1.0,
                base=0, pattern=[[-1, P]], channel_multiplier=1)

            for layer in (1, 2):
                if layer == 2:
                    for s_ in (0, 1):
                        nc.sync.dma_start(idx[s_][:], ixp[(2, s_)][:])
                gtiles = {0: [None] * n_calls[0], 1: [None] * n_calls[1]}
                issued = {0: -1, 1: -1}

                def issue_call(s_, k, layer=layer, gtiles=gtiles, issued=issued,
                               t_reg=t_reg):
                    while issued[s_] < k:
                        kk = issued[s_] + 1
                        g = gp.tile([P, TCALL, F], bf16, tag=f"g{s_}",
                                    name=f"g_{layer}_{s_}_{kk}")
                        if layer == 1 and kk < 2:
                            nc.vector.memset(g[:], 0.0)
                        if layer == 1:
                            tab = xfull[0:half1, :] if s_ == 0 else \
                                  xfull[half1:n_nodes, :]
                        else:
                            tab = ag_out[0:half2, :] if s_ == 0 else \
                                  ag_out[half2:NC_CORES * cols, :]
                        nvalid = min(t_reg[s_] * P - kk * TCALL * P, TCALL * P)
                        nc.gpsimd.dma_gather(
                            g[:], tab,
                            idx[s_][:, kk * TCALL * 8:(kk + 1) * TCALL * 8],
                            num_idxs=TCALL * P, num_idxs_reg=nvalid,
                            elem_size=F, single_packet=False,
                            queue_num=2 * s_ + (kk % 2),
                        )
                        gtiles[s_][kk] = g
                        issued[s_] = kk

                wt = w1t if layer == 1 else w2t
                bt = b1t if layer == 1 else b2t
                for w in range(n_win):
                    pa = [psA.tile([P, P], f32, tag=f"agg{h}",
                                   name=f"agg_{layer}_{w}_{h}") for h in (0, 1)]
                    for h in (0, 1):
                        nc.vector.memset(pa[h][:], 0.0)
                    blocks = win_blocks[w]
                    for j, (s_, t) in enumerate(blocks):
                        issue_call(s_, t // TCALL)
                        g = gtiles[s_][t // TCALL]
                        tl = t % TCALL
                        a, b_ = int(c0[s_][t] - w * P), int(span[s_][t])
                        for h in (0, 1):
                            nc.tensor.matmul(
                                pa[h][:, a:a + b_],
                                lhsT=g[:, tl, h * P:(h + 1) * P],
                                rhs=bv[:, int(boff[s_][t]):int(boff[s_][t] + b_)],
                                start=False,
                                stop=(j == len(blocks) - 1),
                                skip_group_check=True,
                            )
                    m_sb = [mp.tile([P, P], bf16, tag=f"m{h}",
                                    name=f"m_{layer}_{w}_{h}") for h in (0, 1)]
                    for h in (0, 1):
                        nc.vector.tensor_tensor(
                            m_sb[h][:], pa[h][:], dinvt[:, w * P:(w + 1) * P],
                            mybir.AluOpType.mult)
                    pw = [psW.tile([P, P], f32, tag=f"w{h}",
                                   name=f"pw_{layer}_{w}_{h}") for h in (0, 1)]
                    for oh in (0, 1):
                        for kh in (0, 1):
                            nc.tensor.matmul(
                                pw[oh][:],
                                lhsT=wt[:, kh, oh * P:(oh + 1) * P],
                                rhs=m_sb[kh][:],
                                start=(kh == 0), stop=(kh == 1),
                            )
                    xt = xp.tile([P, 2, P], bf16, tag="xt", name=f"xt_{layer}_{w}")
                    for oh in (0, 1):
                        nc.vector.tensor_scalar(
                            xt[:, oh, :], pw[oh][:], bt[:, oh:oh + 1], 0.0,
                            mybir.AluOpType.add, mybir.AluOpType.max)
                    if layer == 1:
                        xn = xp.tile([P, F], bf16, tag="xn", name=f"xn_{w}")
                        for oh in (0, 1):
                            pt = psM.tile([P, P], bf16, tag="misc",
                                          name=f"pt_{w}_{oh}")
                            nc.tensor.transpose(pt[:], xt[:, oh, :], identf[:])
                            nc.vector.tensor_copy(xn[:, oh * P:(oh + 1) * P], pt[:])
                        nc.sync.dma_start(ag_in[w * P:(w + 1) * P, :], xn[:])
                        for ci in range(AG_CHUNKS):
                            if w + 1 == wb[ci + 1] and wb[ci] < wb[ci + 1]:
                                lo, hi = wb[ci] * P, wb[ci + 1] * P
                                nc.gpsimd.collective_compute(
                                    "AllGather", mybir.AluOpType.bypass,
                                    replica_groups=[list(range(NC_CORES))],
                                    ins=[ag_in[lo:hi, :].opt()],
                                    outs=[agc[ci][:].opt()],
                                )
                                nr = hi - lo
                                for r in range(NC_CORES):
                                    nc.scalar.dma_start(
                                        ag_out[r * cols + lo:r * cols + hi, :],
                                        agc[ci][r * nr:(r + 1) * nr, :])
                    else:
                        ph = psM.tile([2, P], f32, tag="misc", name=f"ph_{w}")
                        for kh in (0, 1):
                            nc.tensor.matmul(
                                ph[:], lhsT=wact[:, kh, :], rhs=xt[:, kh, :],
                                start=(kh == 0), stop=(kh == 1))
                        nc.vector.tensor_scalar(
                            outt[:, w * P:(w + 1) * P], ph[:], bact[:, 0:1], None,
                            mybir.AluOpType.add)

            nc.sync.dma_start(outp[:], outt[:])
    nc.compile()
    return nc


def kernel(x, edge_index, W1, b1, W2, b2, Wa, ba, Wc, bc):
    global LAST_EXEC_NS
    x = np.asarray(x)
    edge_index = np.asarray(edge_index)
    n_nodes = x.shape[0]
    src = edge_index[0].astype(np.int64)
    dst = edge_index[1].astype(np.int64)

    plan = _plan(n_nodes, src, dst)
    dinv = (1.0 / np.sqrt(1.0 + plan["deg_in"].astype(np.float64))).astype(np.float32)
    cores = _per_core_data(plan, n_nodes, src, dst, dinv)
    nc = _build(plan, n_nodes)

    x_bf = np.ascontiguousarray(x.astype(BF16))
    W1 = np.asarray(W1, dtype=np.float32); W2 = np.asarray(W2, dtype=np.float32)
    Wa = np.asarray(Wa, dtype=np.float32).reshape(F, 1)
    Wc = np.asarray(Wc, dtype=np.float32).reshape(F, 1)

    def pack_w(w):
        m = w.shape[1]
        return np.ascontiguousarray(
            w.reshape(2, P, m).transpose(1, 0, 2).reshape(P, 2 * m).astype(BF16))

    w1p, w2p = pack_w(W1), pack_w(W2)
    wacp = pack_w(np.concatenate([Wa, Wc], axis=1))

    def pack_b(b):
        return np.ascontiguousarray(np.asarray(b, np.float32).reshape(2, P).T)

    b1p, b2p = pack_b(b1), pack_b(b2)
    bacp = np.array([[float(np.asarray(ba).reshape(-1)[0])],
                     [float(np.asarray(bc).reshape(-1)[0])]], dtype=np.float32)

    in_maps = []
    for c in range(NC_CORES):
        d = cores[c]
        in_maps.append({
            "xfull": x_bf, "w1p": w1p, "w2p": w2p, "wacp": wacp,
            "b1p": b1p, "b2p": b2p, "bacp": bacp,
            "dinvp": d["dinvb"], "bvp": d["bvals"],
            "iA1": d["iA1"], "iB1": d["iB1"], "iA2": d["iA2"], "iB2": d["iB2"],
        })

    trace = bool(int(os.environ.get("KERNEL_TRACE", "0")))
    res = run_bass_kernel_spmd(nc, in_maps, core_ids=list(range(NC_CORES)),
                               trace=trace)
    LAST_EXEC_NS = res.exec_time_ns

    logits = np.zeros(n_nodes, dtype=np.float32)
    value = np.zeros((n_nodes, 1), dtype=np.float32)
    for c in range(NC_CORES):
        o = res.results[c]["out"]
        ids = plan["col_to_node"][c]
        logits[ids] = o[0, : len(ids)]
        value[ids, 0] = o[1, : len(ids)]
    return logits, value
